# revision 38
# baseline (speedup 1.0000x reference)
"""CapsuleLayer (dynamic routing) Trainium2 kernel, SPMD over 8 NeuronCores.

Sharding: input-capsule axis (IN_CAPS=512 -> 64 per core). W and u_hat are
i-sharded; the bij,bijd->bjd contraction is completed with an AllReduce of
s-partials once per routing iteration (ReduceScatter on the last).

Per-core layout (i_local = i2*32 + i1, i2 in {0,1}):
  u_hat SBUF [p=(i2*64+b), (d, i1, j)] bf16 -- 128 partitions x 16384
  b/c logits [p, (i1, j)], s [p, (d, j)].

Phase 1 (per i): u_hat_i[b, dj] = xT_i.T @ W_i on the PE (K=128, M=64,
N=512), all in bf16. It finishes (~50us) inside the collective-stack init
window, so only the routing phase is latency-critical.

Schedule relative to the v1 kernel (252us -> ~215-235us measured; the
collective-stack init varies 26-99us run to run and dominates the spread):
- A 128 B warmup AllReduce is issued first: it synchronizes the 8 cores
  and absorbs the CC-stack init plus the ~11us first-collective premium
  while phase 1 runs. Without it the first real AllReduce costs ~48us.
- s-partials are AllReduced in bf16 (they only feed the routing logits
  and the squash scale; measured error impact <1e-3). The two partition
  halves are folded during staging by a plain + CCE-accumulate gpsimd
  DMA pair, so each AllReduce ships 32 KB.
- Squash is deferred: the agreement update uses the raw AllReduced s and
  the squash scale alpha is folded into the logit increment afterwards
  (outputs = alpha*s, so <outputs,u> = alpha*<s,u>). The ACT sqrt for
  alpha hides under the big b-update multiply.
- Every s is AllReduced per j-half: the b-update for half 0 runs under
  half 1's AllReduce. Pulls are two plain HWDGE DMAs into both partition
  halves (a single stride-0 double-read DMA posts its completion
  semaphore ~8us late, as do SWDGE casting pulls — avoid both).
- The final iteration uses a bf16 ReduceScatter: core k receives batch
  rows [8k:8k+8), squashes locally on 8 partitions, and the host
  concatenates the per-core outputs. A dummy FD-1 sqrt pinned after the
  last softmax exp (via a junk external output so DCE keeps it) preloads
  the ACT sqrt table set off the post-ReduceScatter critical path.
"""

import numpy as np

N_CORES = 8
B = 64
IN_CAPS = 512
IN_DIM = 128
N_CAPS = 16
OUT_DIM = 32
I_LOC = IN_CAPS // N_CORES          # 64 input capsules per core
I1 = 32                             # i_local = i2*32 + i1
JD = N_CAPS * OUT_DIM               # 512
B_LOC = B // N_CORES                # 8 batch rows per core after the final RS
EPS = 1e-7
GRP = 4                             # i's per W-DMA/PSUM group
NGRP = I_LOC // GRP                 # 16

# Toggled by test.py for profiling runs.
TRACE = False
TRACE_DIR = None

_cache = {}


def _tree_i1(nc, tmp, j0, j1, s_outs, first_from_u=None):
    """Reduce tmp[:, :, i1, j0:j1] over i1 into the s_outs tiles (each
    [128, OUT_DIM, w] f32, consecutive j-slices covering j0:j1).

    All levels are in-place contiguous bf16 adds (2x DVE mode); the final
    level writes f32, split per s_out so each stage DMA reads a contiguous
    tile. If first_from_u is given, the first halving reads the two u_hat
    i1-halves directly (uniform-c iteration 0)."""
    if first_from_u is not None:
        u = first_from_u
        nc.vector.tensor_add(
            tmp[:, :, : I1 // 2, j0:j1],
            u[:, :, : I1 // 2, j0:j1],
            u[:, :, I1 // 2:, j0:j1],
        )
    else:
        nc.vector.tensor_add(
            tmp[:, :, : I1 // 2, j0:j1],
            tmp[:, :, : I1 // 2, j0:j1],
            tmp[:, :, I1 // 2:, j0:j1],
        )
    w = I1 // 2
    while w > 2:
        nc.vector.tensor_add(
            tmp[:, :, : w // 2, j0:j1],
            tmp[:, :, : w // 2, j0:j1],
            tmp[:, :, w // 2: w, j0:j1],
        )
        w //= 2
    jc = j0
    for s_out in s_outs:
        jw = s_out.shape[2]
        nc.vector.tensor_add(
            s_out[:], tmp[:, :, 0, jc:jc + jw], tmp[:, :, 1, jc:jc + jw]
        )
        jc += jw


def _tree_d(nc, tmp, j0, j1, q_out):
    """Reduce tmp[:, d, :, j0:j1] over d into q_out [128, I1, j1-j0] f32."""
    w = OUT_DIM
    while w > 2:
        nc.vector.tensor_add(
            tmp[:, : w // 2, :, j0:j1],
            tmp[:, : w // 2, :, j0:j1],
            tmp[:, w // 2: w, :, j0:j1],
        )
        w //= 2
    nc.vector.tensor_add(q_out[:], tmp[:, 0, :, j0:j1], tmp[:, 1, :, j0:j1])


def _emit(tc, xT, wT, out, junk, num_routing):
    from contextlib import ExitStack, nullcontext

    from concourse import mybir

    nc = tc.nc
    f32 = mybir.dt.float32
    bf16 = mybir.dt.bfloat16
    ADD = mybir.AluOpType.add
    MUL = mybir.AluOpType.mult
    GROUPS = [list(range(N_CORES))]
    ctx = ExitStack()
    singles = ctx.enter_context(tc.tile_pool(name="singles", bufs=1))
    wpool = ctx.enter_context(tc.tile_pool(name="wpool", bufs=4))
    pspool = ctx.enter_context(tc.tile_pool(name="pspool", bufs=2, space="PSUM"))
    dram = ctx.enter_context(tc.tile_pool(name="dram", bufs=8, space="DRAM"))

    # One tiny warmup collective issued first: it synchronizes the 8 cores
    # and absorbs both the collective-stack init (~64us) and the
    # first-collective premium (~12us) while phase 1 runs. Without it the
    # first real AllReduce pays ~48us.
    warm_in = dram.tile([1, 32], f32)
    warm_out = dram.tile([1, 32], f32)
    nc.gpsimd.collective_compute(
        "AllReduce",
        mybir.AluOpType.add,
        replica_groups=[list(range(N_CORES))],
        ins=[warm_in.opt()],
        outs=[warm_out.opt()],
    )

    # ---- phase 1: u_hat = einsum over k, per local capsule i ----
    xsb = singles.tile([IN_DIM, I_LOC, B], bf16)            # [k, i, b]
    u_hat = singles.tile([128, OUT_DIM, I1, N_CAPS], bf16)  # [(i2,b), d, i1, j]

    XCH = I_LOC // 4
    for g in range(NGRP):
        i2 = (g * GRP) // I1
        i1g = (g * GRP) % I1
        if g < 4:
            q = g
            nc.sync.dma_start(
                xsb[:, q * XCH:(q + 1) * XCH, :],
                xT[:, q * XCH:(q + 1) * XCH, :],
            )
        wtile = wpool.tile([IN_DIM, GRP, OUT_DIM, N_CAPS], bf16)
        nc.sync.dma_start(wtile[:], wT[g])
        ps = pspool.tile([128, GRP, OUT_DIM, N_CAPS], f32)
        for t in range(GRP):
            i = g * GRP + t
            nc.tensor.matmul(
                ps[i2 * B:(i2 + 1) * B, t], xsb[:, i, :], wtile[:, t],
                start=True, stop=True,
            )
        dst = u_hat[i2 * B:(i2 + 1) * B, :, i1g:i1g + GRP, :].transpose(
            [0, 2, 1, 3]
        )
        src = ps[i2 * B:(i2 + 1) * B]
        if g % 2 == 0:
            nc.vector.tensor_copy(out=dst, in_=src)
        else:
            nc.scalar.copy(out=dst, in_=src)

    # ---- phase 2: routing ----
    tmp = singles.tile([128, OUT_DIM, I1, N_CAPS], bf16)
    b_log = singles.tile([128, I1, N_CAPS], f32)
    c_t = singles.tile([128, I1, N_CAPS], bf16)
    cexp = singles.tile([128, I1, N_CAPS], f32)
    csum = singles.tile([128, I1], f32)
    eps_t = singles.tile([128, 1], f32)
    nc.vector.memset(eps_t[:], EPS)

    R = num_routing
    JH = N_CAPS // 2

    def stage_and_reduce(s_half, kind, jw):
        """Fold the two partition halves into one DRAM buffer (a plain +
        CCE-accumulate gpsimd DMA pair) and run the collective at s_half's
        dtype (bf16 for the mid-iteration AllReduces, f32/bf16 for the
        final ReduceScatter). Returns the collective's output tile."""
        dt = s_half.dtype
        cc_in = dram.tile([B, OUT_DIM, jw], dt)
        nc.gpsimd.dma_start(cc_in[:], s_half[0:B])
        nc.gpsimd.dma_start(cc_in[:], s_half[B:2 * B], accum_op=ADD)
        if kind == "AllReduce":
            cc_out = dram.tile([B, OUT_DIM, jw], dt, addr_space="Shared")
        else:
            cc_out = dram.tile([B_LOC, OUT_DIM, jw], dt)
        nc.gpsimd.collective_compute(
            kind, ADD, replica_groups=GROUPS,
            ins=[cc_in.opt()], outs=[cc_out.opt()],
        )
        return cc_out

    def pull_bf(cc_out, jw):
        """Pull an AllReduced bf16 buffer into both partition halves (two
        plain HWDGE DMAs on separate queues; a single stride-0 double-read
        DMA posts its completion semaphore ~8us late)."""
        s_b = singles.tile([128, OUT_DIM, jw], bf16, name="s_b")
        nc.sync.dma_start(s_b[0:B], cc_out[:])
        nc.scalar.dma_start(s_b[B:2 * B], cc_out[:])
        return s_b

    def alpha_head(s_f, jw, inv):
        """ss (sum over d of (inv*s)^2) then kick the ACT sqrt. Returns
        (ss, t1) tiles; finish with alpha_tail."""
        p = s_f.shape[0]
        sq = singles.tile([p, OUT_DIM, jw], f32)
        ss = singles.tile([p, jw], f32)
        t1 = singles.tile([p, jw], f32)
        nc.vector.scalar_tensor_tensor(
            sq[:], s_f[:], inv * inv, s_f[:], MUL, MUL,
        )
        nc.vector.reduce_sum(
            out=ss[:], in_=sq.transpose([0, 2, 1]), axis=mybir.AxisListType.X
        )
        nc.scalar.activation(
            out=t1[:], in_=ss[:], func=mybir.ActivationFunctionType.Sqrt,
            bias=eps_t[0:p], scale=1.0,
        )
        return ss, t1

    def alpha_tail(ss, t1, inv):
        """alpha = (ss*inv) / ((1+ss)*sqrt(ss+eps)); squash scale incl inv."""
        t2 = singles.tile(list(ss.shape), f32)
        al = singles.tile(list(ss.shape), f32)
        nc.vector.scalar_tensor_tensor(t2[:], ss[:], 1.0, t1[:], ADD, MUL)
        nc.vector.reciprocal(out=t2[:], in_=t2[:])
        nc.vector.scalar_tensor_tensor(al[:], ss[:], inv, t2[:], MUL, MUL)
        return al

    def softmax():
        nc.scalar.activation(
            out=cexp[:], in_=b_log[:], func=mybir.ActivationFunctionType.Exp,
        )
        nc.vector.reduce_sum(
            out=csum[:], in_=cexp[:], axis=mybir.AxisListType.X
        )
        nc.vector.reciprocal(out=csum[:], in_=csum[:])
        nc.vector.tensor_mul(
            c_t[:], cexp[:],
            csum.unsqueeze(2).broadcast_to([128, I1, N_CAPS]),
        )

    def final_tail(cc_out, jw, inv):
        """Per-core tail after ReduceScatter: squash 8 batch rows, write out."""
        s8 = singles.tile([B_LOC, OUT_DIM, jw], cc_out.dtype)
        nc.sync.dma_start(s8[:], cc_out[:])
        ss, t1 = alpha_head(s8, jw, inv)
        al = alpha_tail(ss, t1, inv)
        out_t = singles.tile([B_LOC, jw, OUT_DIM], f32)
        nc.vector.tensor_mul(
            out_t.transpose([0, 2, 1]), s8[:],
            al.unsqueeze(1).broadcast_to([B_LOC, OUT_DIM, jw]),
        )
        nc.sync.dma_start(out[:], out_t[:])

    def bu_half(cc, j0, j1, inv, first):
        """Pull the reduced s for j0:j1 and run the b-update from the raw
        (unsquashed) s; the squash scale alpha is folded into the logit
        increment afterwards."""
        jw = j1 - j0
        s_b = pull_bf(cc, jw)
        # ACT sqrt in alpha_head overlaps the big multiply below
        ss, t1 = alpha_head(s_b, jw, inv)
        binc = singles.tile([128, I1, jw], f32, name="binc")
        nc.vector.tensor_mul(
            tmp[:, :, :, j0:j1], u_hat[:, :, :, j0:j1],
            s_b.unsqueeze(2).broadcast_to([128, OUT_DIM, I1, jw]),
        )
        al = alpha_tail(ss, t1, inv)
        _tree_d(nc, tmp, j0, j1, binc)
        if first:
            # b starts at 0, so b_log = alpha*binc directly
            nc.vector.tensor_mul(
                b_log[:, :, j0:j1], binc[:],
                al.unsqueeze(1).broadcast_to([128, I1, jw]),
            )
        else:
            tt = singles.tile([128, I1, jw], f32, name="tt")
            nc.vector.tensor_mul(
                tt[:], binc[:], al.unsqueeze(1).broadcast_to([128, I1, jw])
            )
            nc.vector.tensor_add(
                b_log[:, :, j0:j1], b_log[:, :, j0:j1], tt[:]
            )

    # --- iteration 0: c uniform; s0 = sum_i u_hat, scale folded into alpha
    if R == 1:
        s_half0 = singles.tile([128, OUT_DIM, N_CAPS], f32)
        _tree_i1(nc, tmp, 0, N_CAPS, [s_half0], first_from_u=u_hat)
        cc0 = stage_and_reduce(s_half0, "ReduceScatter", N_CAPS)
        final_tail(cc0, N_CAPS, 1.0 / N_CAPS)
        ctx.close()
        return

    # r0: the tree runs during the CC-init window; AllReduce per j-half so
    # the b-update for half 0 overlaps half 1's AllReduce.
    sh0 = singles.tile([128, OUT_DIM, JH], bf16)
    sh1 = singles.tile([128, OUT_DIM, JH], bf16)
    _tree_i1(nc, tmp, 0, N_CAPS, [sh0, sh1], first_from_u=u_hat)
    cc_h0 = stage_and_reduce(sh0, "AllReduce", JH)
    cc_h1 = stage_and_reduce(sh1, "AllReduce", JH)
    inv0 = 1.0 / N_CAPS
    bu_half(cc_h0, 0, JH, inv0, first=True)
    bu_half(cc_h1, JH, N_CAPS, inv0, first=True)
    softmax()

    # --- middle iterations: s and b-update pipelined in j-halves ---
    for r in range(1, R - 1):
        cc_h = []
        for h in range(2):
            j0, j1 = h * JH, (h + 1) * JH
            prio = tc.high_priority() if h == 0 else nullcontext()
            with prio:
                nc.vector.tensor_mul(
                    tmp[:, :, :, j0:j1], u_hat[:, :, :, j0:j1],
                    c_t[:, :, j0:j1].unsqueeze(1).broadcast_to(
                        [128, OUT_DIM, I1, JH]
                    ),
                )
                s_half = singles.tile(
                    [128, OUT_DIM, JH], bf16, name=f"s_half_{r}_{h}"
                )
                _tree_i1(nc, tmp, j0, j1, [s_half])
            cc_h.append(stage_and_reduce(s_half, "AllReduce", JH))
        bu_half(cc_h[0], 0, JH, 1.0, first=False)
        bu_half(cc_h[1], JH, N_CAPS, 1.0, first=False)
        softmax()

    # --- final iteration: s, ReduceScatter, local squash on 8 rows ---
    # Dummy FD-1 sqrt written to a junk external output (so DCE keeps it).
    # It reads cexp so it schedules AFTER the last softmax exp, which makes
    # walrus reload the ACT sqrt table set during the final multiply/tree
    # instead of on the post-ReduceScatter critical path.
    dummy = singles.tile([1, 1], f32)
    nc.scalar.activation(
        out=dummy[:], in_=cexp[0:1, 0:1, 0:1],
        func=mybir.ActivationFunctionType.Sqrt,
        bias=eps_t[0:1], scale=1.0,
    )
    nc.scalar.dma_start(junk[:], dummy[:])
    nc.vector.tensor_mul(
        tmp[:], u_hat[:],
        c_t.unsqueeze(1).broadcast_to([128, OUT_DIM, I1, N_CAPS]),
    )
    s_half = singles.tile([128, OUT_DIM, N_CAPS], bf16)
    _tree_i1(nc, tmp, 0, N_CAPS, [s_half])
    cc_out = stage_and_reduce(s_half, "ReduceScatter", N_CAPS)
    final_tail(cc_out, N_CAPS, 1.0)

    ctx.close()


def _build(num_routing):
    import concourse.bacc as bacc
    import concourse.tile as tile
    from concourse import mybir

    nc = bacc.Bacc(
        "TRN2", target_bir_lowering=False, debug=False, num_devices=N_CORES,
        dynamic_dma_scratch_size=512,
    )
    f32 = mybir.dt.float32
    bf16 = mybir.dt.bfloat16
    xT = nc.dram_tensor("xT", [IN_DIM, I_LOC, B], bf16, kind="ExternalInput")
    wT = nc.dram_tensor(
        "wT", [NGRP, IN_DIM, GRP, OUT_DIM, N_CAPS], bf16, kind="ExternalInput"
    )
    out = nc.dram_tensor(
        "out", [B_LOC, N_CAPS, OUT_DIM], f32, kind="ExternalOutput"
    )
    junk = nc.dram_tensor("junk", [1, 1], f32, kind="ExternalOutput")
    with tile.TileContext(nc) as tc:
        _emit(tc, xT, wT, out, junk, num_routing)
    nc.compile()
    return nc


def kernel(inputs, W, num_routing):
    import ml_dtypes

    from concourse.bass_utils import run_bass_kernel_spmd

    R = int(num_routing)
    assert R >= 1
    if R not in _cache:
        _cache[R] = _build(R)
    nc = _cache[R]

    bf = ml_dtypes.bfloat16
    inputs = np.ascontiguousarray(np.asarray(inputs, dtype=np.float32))
    W = np.asarray(W, dtype=np.float32)

    in_maps = []
    for c in range(N_CORES):
        lo, hi = c * I_LOC, (c + 1) * I_LOC
        xT_c = np.ascontiguousarray(
            inputs[:, lo:hi, :].transpose(2, 1, 0).astype(bf)
        )
        # [i,j,k,d] -> group-blocked [g, k, t, d, j] so each group DMA is one
        # contiguous block and PSUM columns come out in (d, j) order
        wT_c = np.ascontiguousarray(
            W[lo:hi]
            .reshape(NGRP, GRP, N_CAPS, IN_DIM, OUT_DIM)
            .transpose(0, 3, 1, 4, 2)
            .astype(bf)
        )
        in_maps.append({"xT": xT_c, "wT": wT_c})

    kwargs = {}
    if TRACE:
        kwargs["trace"] = True
        if TRACE_DIR:
            kwargs["tmpdir"] = TRACE_DIR
    res = None
    for attempt in range(3):
        try:
            res = run_bass_kernel_spmd(
                nc, in_maps, core_ids=list(range(N_CORES)), **kwargs
            )
            break
        except Exception:
            if attempt == 2:
                raise
            import time
            time.sleep(5)
    if TRACE:
        kernel.last_exec_time_ns = res.exec_time_ns
        kernel.last_results = res
    # ReduceScatter leaves batch rows [8c:8c+8) on core c; reassemble.
    return np.concatenate(
        [np.asarray(res.results[c]["out"], dtype=np.float32)
         for c in range(N_CORES)],
        axis=0,
    )


# revision 42
# speedup vs baseline: 1.0555x; 1.0555x over previous
"""CapsuleLayer (dynamic routing) Trainium2 kernel, SPMD over 8 NeuronCores.

Sharding: input-capsule axis (IN_CAPS=512 -> 64 per core). W and u_hat are
i-sharded; the bij,bijd->bjd contraction is completed with an AllReduce of
s-partials once per routing iteration (ReduceScatter on the last).

Per-core layout (i_local = i2*32 + i1, i2 in {0,1}):
  u_hat SBUF [p=(i2*64+b), (d, i1, j)] bf16 -- 128 partitions x 16384
  b/c logits [p, (i1, j)], s [p, (d, j)].

Phase 1 (per i): u_hat_i[b, dj] = xT_i.T @ W_i on the PE (K=128, M=64,
N=512), all in bf16. It finishes (~50us) inside the collective-stack init
window, so only the routing phase is latency-critical.

Schedule relative to the v1 kernel (252us -> ~215-235us measured; the
collective-stack init varies 26-99us run to run and dominates the spread):
- A 128 B warmup AllReduce is issued first: it synchronizes the 8 cores
  and absorbs the CC-stack init plus the ~11us first-collective premium
  while phase 1 runs. Without it the first real AllReduce costs ~48us.
- s-partials are AllReduced in bf16 (they only feed the routing logits
  and the squash scale; measured error impact <1e-3). The two partition
  halves are folded during staging by a plain + CCE-accumulate gpsimd
  DMA pair, so each AllReduce ships 32 KB.
- Squash is deferred: the agreement update uses the raw AllReduced s and
  the squash scale alpha is folded into the logit increment afterwards
  (outputs = alpha*s, so <outputs,u> = alpha*<s,u>). The ACT sqrt for
  alpha hides under the big b-update multiply.
- Every s is AllReduced per j-half: the b-update for half 0 runs under
  half 1's AllReduce. Pulls are two plain HWDGE DMAs into both partition
  halves (a single stride-0 double-read DMA posts its completion
  semaphore ~8us late, as do SWDGE casting pulls — avoid both).
- The final iteration uses a bf16 ReduceScatter: core k receives batch
  rows [8k:8k+8), squashes locally on 8 partitions, and the host
  concatenates the per-core outputs. A dummy FD-1 sqrt pinned after the
  last softmax exp (via a junk external output so DCE keeps it) preloads
  the ACT sqrt table set off the post-ReduceScatter critical path.
"""

import numpy as np

N_CORES = 8
B = 64
IN_CAPS = 512
IN_DIM = 128
N_CAPS = 16
OUT_DIM = 32
I_LOC = IN_CAPS // N_CORES          # 64 input capsules per core
I1 = 32                             # i_local = i2*32 + i1
JD = N_CAPS * OUT_DIM               # 512
B_LOC = B // N_CORES                # 8 batch rows per core after the final RS
EPS = 1e-7
GRP = 4                             # i's per W-DMA/PSUM group
NGRP = I_LOC // GRP                 # 16

# Toggled by test.py for profiling runs.
TRACE = False
TRACE_DIR = None

_cache = {}


def _tree_i1(nc, tmp, j0, j1, s_outs, first_from_u=None):
    """Reduce tmp[:, :, i1, j0:j1] over i1 into the s_outs tiles (each
    [128, OUT_DIM, w] f32, consecutive j-slices covering j0:j1).

    All levels are in-place contiguous bf16 adds (2x DVE mode); the final
    level writes f32, split per s_out so each stage DMA reads a contiguous
    tile. If first_from_u is given, the first halving reads the two u_hat
    i1-halves directly (uniform-c iteration 0)."""
    if first_from_u is not None:
        u = first_from_u
        nc.vector.tensor_add(
            tmp[:, :, : I1 // 2, j0:j1],
            u[:, :, : I1 // 2, j0:j1],
            u[:, :, I1 // 2:, j0:j1],
        )
    else:
        nc.vector.tensor_add(
            tmp[:, :, : I1 // 2, j0:j1],
            tmp[:, :, : I1 // 2, j0:j1],
            tmp[:, :, I1 // 2:, j0:j1],
        )
    w = I1 // 2
    while w > 2:
        nc.vector.tensor_add(
            tmp[:, :, : w // 2, j0:j1],
            tmp[:, :, : w // 2, j0:j1],
            tmp[:, :, w // 2: w, j0:j1],
        )
        w //= 2
    jc = j0
    for s_out in s_outs:
        jw = s_out.shape[2]
        nc.vector.tensor_add(
            s_out[:], tmp[:, :, 0, jc:jc + jw], tmp[:, :, 1, jc:jc + jw]
        )
        jc += jw


def _tree_d(nc, tmp, j0, j1, q_out):
    """Reduce tmp[:, d, :, j0:j1] over d into q_out [128, I1, j1-j0] f32."""
    w = OUT_DIM
    while w > 2:
        nc.vector.tensor_add(
            tmp[:, : w // 2, :, j0:j1],
            tmp[:, : w // 2, :, j0:j1],
            tmp[:, w // 2: w, :, j0:j1],
        )
        w //= 2
    nc.vector.tensor_add(q_out[:], tmp[:, 0, :, j0:j1], tmp[:, 1, :, j0:j1])


def _emit(tc, xT, wT, out, junk, num_routing):
    from contextlib import ExitStack, nullcontext

    from concourse import mybir

    nc = tc.nc
    f32 = mybir.dt.float32
    bf16 = mybir.dt.bfloat16
    ADD = mybir.AluOpType.add
    MUL = mybir.AluOpType.mult
    GROUPS = [list(range(N_CORES))]
    ctx = ExitStack()
    singles = ctx.enter_context(tc.tile_pool(name="singles", bufs=1))
    wpool = ctx.enter_context(tc.tile_pool(name="wpool", bufs=4))
    pspool = ctx.enter_context(tc.tile_pool(name="pspool", bufs=2, space="PSUM"))
    dram = ctx.enter_context(tc.tile_pool(name="dram", bufs=8, space="DRAM"))

    # One tiny warmup collective issued first: it synchronizes the 8 cores
    # and absorbs both the collective-stack init (~64us) and the
    # first-collective premium (~12us) while phase 1 runs. Without it the
    # first real AllReduce pays ~48us.
    warm_in = dram.tile([1, 32], f32)
    warm_out = dram.tile([1, 32], f32)
    nc.gpsimd.collective_compute(
        "AllReduce",
        mybir.AluOpType.add,
        replica_groups=[list(range(N_CORES))],
        ins=[warm_in.opt()],
        outs=[warm_out.opt()],
    )

    # ---- phase 1: u_hat = einsum over k, per local capsule i ----
    xsb = singles.tile([IN_DIM, I_LOC, B], bf16)            # [k, i, b]
    u_hat = singles.tile([128, OUT_DIM, I1, N_CAPS], bf16)  # [(i2,b), d, i1, j]

    XCH = I_LOC // 4
    for g in range(NGRP):
        i2 = (g * GRP) // I1
        i1g = (g * GRP) % I1
        if g < 4:
            q = g
            nc.sync.dma_start(
                xsb[:, q * XCH:(q + 1) * XCH, :],
                xT[:, q * XCH:(q + 1) * XCH, :],
            )
        wtile = wpool.tile([IN_DIM, GRP, OUT_DIM, N_CAPS], bf16)
        nc.sync.dma_start(wtile[:], wT[g])
        ps = pspool.tile([128, GRP, OUT_DIM, N_CAPS], f32)
        for t in range(GRP):
            i = g * GRP + t
            nc.tensor.matmul(
                ps[i2 * B:(i2 + 1) * B, t], xsb[:, i, :], wtile[:, t],
                start=True, stop=True,
            )
        dst = u_hat[i2 * B:(i2 + 1) * B, :, i1g:i1g + GRP, :].transpose(
            [0, 2, 1, 3]
        )
        src = ps[i2 * B:(i2 + 1) * B]
        if g % 2 == 0:
            nc.vector.tensor_copy(out=dst, in_=src)
        else:
            nc.scalar.copy(out=dst, in_=src)

    # ---- phase 2: routing ----
    tmp = singles.tile([128, OUT_DIM, I1, N_CAPS], bf16)
    b_log = singles.tile([128, I1, N_CAPS], f32)
    c_t = singles.tile([128, I1, N_CAPS], bf16)
    cexp = singles.tile([128, I1, N_CAPS], f32)
    csum = singles.tile([128, I1], f32)
    eps_t = singles.tile([128, 1], f32)
    nc.vector.memset(eps_t[:], EPS)

    R = num_routing
    JH = N_CAPS // 2

    def stage_and_reduce(s_half, kind, jw):
        """Fold the two partition halves into one DRAM buffer (a plain +
        CCE-accumulate gpsimd DMA pair) and run the collective at s_half's
        dtype (bf16 for the mid-iteration AllReduces, f32/bf16 for the
        final ReduceScatter). Returns the collective's output tile."""
        dt = s_half.dtype
        cc_in = dram.tile([B, OUT_DIM, jw], dt)
        nc.gpsimd.dma_start(cc_in[:], s_half[0:B])
        nc.gpsimd.dma_start(cc_in[:], s_half[B:2 * B], accum_op=ADD)
        if kind == "AllReduce":
            cc_out = dram.tile([B, OUT_DIM, jw], dt, addr_space="Shared")
        else:
            cc_out = dram.tile([B_LOC, OUT_DIM, jw], dt)
        nc.gpsimd.collective_compute(
            kind, ADD, replica_groups=GROUPS,
            ins=[cc_in.opt()], outs=[cc_out.opt()],
        )
        return cc_out

    def pull_bf(cc_out, jw):
        """Pull an AllReduced bf16 buffer into both partition halves (two
        plain HWDGE DMAs on separate queues; a single stride-0 double-read
        DMA posts its completion semaphore ~8us late)."""
        s_b = singles.tile([128, OUT_DIM, jw], bf16, name="s_b")
        nc.sync.dma_start(s_b[0:B], cc_out[:])
        nc.scalar.dma_start(s_b[B:2 * B], cc_out[:])
        return s_b

    def alpha_head(s_f, jw, inv):
        """ss (sum over d of (inv*s)^2) then kick the ACT sqrt. Returns
        (ss, t1) tiles; finish with alpha_tail."""
        p = s_f.shape[0]
        sq = singles.tile([p, OUT_DIM, jw], f32)
        ss = singles.tile([p, jw], f32)
        t1 = singles.tile([p, jw], f32)
        nc.vector.scalar_tensor_tensor(
            sq[:], s_f[:], inv * inv, s_f[:], MUL, MUL,
        )
        nc.vector.reduce_sum(
            out=ss[:], in_=sq.transpose([0, 2, 1]), axis=mybir.AxisListType.X
        )
        nc.scalar.activation(
            out=t1[:], in_=ss[:], func=mybir.ActivationFunctionType.Sqrt,
            bias=eps_t[0:p], scale=1.0,
        )
        return ss, t1

    def alpha_tail(ss, t1, inv):
        """alpha = (ss*inv) / ((1+ss)*sqrt(ss+eps)); squash scale incl inv."""
        t2 = singles.tile(list(ss.shape), f32)
        al = singles.tile(list(ss.shape), f32)
        nc.vector.scalar_tensor_tensor(t2[:], ss[:], 1.0, t1[:], ADD, MUL)
        nc.vector.reciprocal(out=t2[:], in_=t2[:])
        nc.vector.scalar_tensor_tensor(al[:], ss[:], inv, t2[:], MUL, MUL)
        return al

    def softmax():
        # exp already computed per j-half inside bu_half
        nc.vector.reduce_sum(
            out=csum[:], in_=cexp[:], axis=mybir.AxisListType.X
        )
        nc.vector.reciprocal(out=csum[:], in_=csum[:])
        nc.vector.tensor_mul(
            c_t[:], cexp[:],
            csum.unsqueeze(2).broadcast_to([128, I1, N_CAPS]),
        )

    def final_tail(cc_out, jw, inv):
        """Per-core tail after ReduceScatter: squash 8 batch rows, write out."""
        s8 = singles.tile([B_LOC, OUT_DIM, jw], cc_out.dtype)
        nc.sync.dma_start(s8[:], cc_out[:])
        ss, t1 = alpha_head(s8, jw, inv)
        al = alpha_tail(ss, t1, inv)
        out_t = singles.tile([B_LOC, jw, OUT_DIM], f32)
        nc.vector.tensor_mul(
            out_t.transpose([0, 2, 1]), s8[:],
            al.unsqueeze(1).broadcast_to([B_LOC, OUT_DIM, jw]),
        )
        nc.sync.dma_start(out[:], out_t[:])

    def bu_half(cc, j0, j1, inv, first):
        """Pull the reduced s for j0:j1 and run the b-update from the raw
        (unsquashed) s; the squash scale alpha is folded into the logit
        increment afterwards."""
        jw = j1 - j0
        s_b = pull_bf(cc, jw)
        # ACT sqrt in alpha_head overlaps the big multiply below
        ss, t1 = alpha_head(s_b, jw, inv)
        binc = singles.tile([128, I1, jw], f32, name="binc")
        nc.vector.tensor_mul(
            tmp[:, :, :, j0:j1], u_hat[:, :, :, j0:j1],
            s_b.unsqueeze(2).broadcast_to([128, OUT_DIM, I1, jw]),
        )
        al = alpha_tail(ss, t1, inv)
        _tree_d(nc, tmp, j0, j1, binc)
        if first:
            # b starts at 0, so b_log = alpha*binc directly
            nc.vector.tensor_mul(
                b_log[:, :, j0:j1], binc[:],
                al.unsqueeze(1).broadcast_to([128, I1, jw]),
            )
        else:
            tt = singles.tile([128, I1, jw], f32, name="tt")
            nc.vector.tensor_mul(
                tt[:], binc[:], al.unsqueeze(1).broadcast_to([128, I1, jw])
            )
            nc.vector.tensor_add(
                b_log[:, :, j0:j1], b_log[:, :, j0:j1], tt[:]
            )
        # per-half softmax exp: half 0's exp hides under half 1's AllReduce
        nc.scalar.activation(
            out=cexp[:, :, j0:j1], in_=b_log[:, :, j0:j1],
            func=mybir.ActivationFunctionType.Exp,
        )

    # --- iteration 0: c uniform; s0 = sum_i u_hat, scale folded into alpha
    if R == 1:
        s_half0 = singles.tile([128, OUT_DIM, N_CAPS], f32)
        _tree_i1(nc, tmp, 0, N_CAPS, [s_half0], first_from_u=u_hat)
        cc0 = stage_and_reduce(s_half0, "ReduceScatter", N_CAPS)
        final_tail(cc0, N_CAPS, 1.0 / N_CAPS)
        ctx.close()
        return

    # r0: the tree runs during the CC-init window; AllReduce per j-half so
    # the b-update for half 0 overlaps half 1's AllReduce.
    sh0 = singles.tile([128, OUT_DIM, JH], bf16)
    sh1 = singles.tile([128, OUT_DIM, JH], bf16)
    _tree_i1(nc, tmp, 0, N_CAPS, [sh0, sh1], first_from_u=u_hat)
    cc_h0 = stage_and_reduce(sh0, "AllReduce", JH)
    cc_h1 = stage_and_reduce(sh1, "AllReduce", JH)
    inv0 = 1.0 / N_CAPS
    bu_half(cc_h0, 0, JH, inv0, first=True)
    bu_half(cc_h1, JH, N_CAPS, inv0, first=True)
    softmax()

    # --- middle iterations: s and b-update pipelined in j-halves ---
    for r in range(1, R - 1):
        cc_h = []
        for h in range(2):
            j0, j1 = h * JH, (h + 1) * JH
            nc.vector.tensor_mul(
                tmp[:, :, :, j0:j1], u_hat[:, :, :, j0:j1],
                c_t[:, :, j0:j1].unsqueeze(1).broadcast_to(
                    [128, OUT_DIM, I1, JH]
                ),
            )
            s_half = singles.tile(
                [128, OUT_DIM, JH], bf16, name=f"s_half_{r}_{h}"
            )
            _tree_i1(nc, tmp, j0, j1, [s_half])
            if h == 0:
                # 1-element copy from half 0's s into half 1's tmp region:
                # the WAW edge forces the scheduler to finish half 0's tree
                # (and so launch its AllReduce) before starting half 1's
                # multiply, instead of interleaving both trees.
                nc.vector.tensor_copy(
                    out=tmp[0:1, 0:1, 0:1, JH:JH + 1],
                    in_=s_half[0:1, 0:1, 0:1],
                )
            cc_h.append(stage_and_reduce(s_half, "AllReduce", JH))
        bu_half(cc_h[0], 0, JH, 1.0, first=False)
        bu_half(cc_h[1], JH, N_CAPS, 1.0, first=False)
        softmax()

    # --- final iteration: s, ReduceScatter, local squash on 8 rows ---
    # Dummy FD-1 sqrt written to a junk external output (so DCE keeps it).
    # It reads cexp so it schedules AFTER the last softmax exp, which makes
    # walrus reload the ACT sqrt table set during the final multiply/tree
    # instead of on the post-ReduceScatter critical path.
    dummy = singles.tile([1, 1], f32)
    nc.scalar.activation(
        out=dummy[:], in_=cexp[0:1, 0:1, N_CAPS - 1:N_CAPS],
        func=mybir.ActivationFunctionType.Sqrt,
        bias=eps_t[0:1], scale=1.0,
    )
    nc.scalar.dma_start(junk[:], dummy[:])
    nc.vector.tensor_mul(
        tmp[:], u_hat[:],
        c_t.unsqueeze(1).broadcast_to([128, OUT_DIM, I1, N_CAPS]),
    )
    s_half = singles.tile([128, OUT_DIM, N_CAPS], bf16)
    _tree_i1(nc, tmp, 0, N_CAPS, [s_half])
    cc_out = stage_and_reduce(s_half, "ReduceScatter", N_CAPS)
    final_tail(cc_out, N_CAPS, 1.0)

    ctx.close()


def _build(num_routing):
    import concourse.bacc as bacc
    import concourse.tile as tile
    from concourse import mybir

    nc = bacc.Bacc(
        "TRN2", target_bir_lowering=False, debug=False, num_devices=N_CORES,
        dynamic_dma_scratch_size=512,
    )
    f32 = mybir.dt.float32
    bf16 = mybir.dt.bfloat16
    xT = nc.dram_tensor("xT", [IN_DIM, I_LOC, B], bf16, kind="ExternalInput")
    wT = nc.dram_tensor(
        "wT", [NGRP, IN_DIM, GRP, OUT_DIM, N_CAPS], bf16, kind="ExternalInput"
    )
    out = nc.dram_tensor(
        "out", [B_LOC, N_CAPS, OUT_DIM], f32, kind="ExternalOutput"
    )
    junk = nc.dram_tensor("junk", [1, 1], f32, kind="ExternalOutput")
    with tile.TileContext(nc) as tc:
        _emit(tc, xT, wT, out, junk, num_routing)
    nc.compile()
    return nc


def kernel(inputs, W, num_routing):
    import ml_dtypes

    from concourse.bass_utils import run_bass_kernel_spmd

    R = int(num_routing)
    assert R >= 1
    if R not in _cache:
        _cache[R] = _build(R)
    nc = _cache[R]

    bf = ml_dtypes.bfloat16
    inputs = np.ascontiguousarray(np.asarray(inputs, dtype=np.float32))
    W = np.asarray(W, dtype=np.float32)

    in_maps = []
    for c in range(N_CORES):
        lo, hi = c * I_LOC, (c + 1) * I_LOC
        xT_c = np.ascontiguousarray(
            inputs[:, lo:hi, :].transpose(2, 1, 0).astype(bf)
        )
        # [i,j,k,d] -> group-blocked [g, k, t, d, j] so each group DMA is one
        # contiguous block and PSUM columns come out in (d, j) order
        wT_c = np.ascontiguousarray(
            W[lo:hi]
            .reshape(NGRP, GRP, N_CAPS, IN_DIM, OUT_DIM)
            .transpose(0, 3, 1, 4, 2)
            .astype(bf)
        )
        in_maps.append({"xT": xT_c, "wT": wT_c})

    kwargs = {}
    if TRACE:
        kwargs["trace"] = True
        if TRACE_DIR:
            kwargs["tmpdir"] = TRACE_DIR
    res = None
    for attempt in range(3):
        try:
            res = run_bass_kernel_spmd(
                nc, in_maps, core_ids=list(range(N_CORES)), **kwargs
            )
            break
        except Exception:
            if attempt == 2:
                raise
            import time
            time.sleep(5)
    if TRACE:
        kernel.last_exec_time_ns = res.exec_time_ns
        kernel.last_results = res
    # ReduceScatter leaves batch rows [8c:8c+8) on core c; reassemble.
    return np.concatenate(
        [np.asarray(res.results[c]["out"], dtype=np.float32)
         for c in range(N_CORES)],
        axis=0,
    )
